# revision 10
# baseline (speedup 1.0000x reference)
"""CompressionAttention Trainium2 kernel (8 NeuronCores, SPMD).

Sharding: core i handles batch b=i//4 and 4 heads hh=i%4 (model-dim slice
hh*256:(hh+1)*256). Heads never interact before out_proj, so each core
computes a partial out-projection for its batch; the host gather sums the
4 partials per batch (tensor-parallel unshard) -- bo is added on-device
as bo/4 by each core.

Algorithm per core (chunked linear attention, chunk T=128):
  w[c,t] = exp(qc_c . k_t)            (max-subtraction dropped: att is
                                       invariant to per-c scaling of w)
  den[c,s]   = cumsum_t<=s w[c,t]
  att[c,s]   = (sum_t<=s w[c,t] (qs_s . k_t)) / den
             = (w_chunk^T (U o G) + SK^T qs) / den   per chunk + state
  p = softmax_c att ; o_s = sum_c p * (sum_t<=s w v)/den
  computed as o = v^T (U o (w p'')) + SV^T p'' scaled by 1/sums,
  p'' = exp(att)/den.

Matmul dtypes: projections/out-proj float32r (tf32-like, ~1.5e-4), the
attention chunk matmuls bf16 with fp32 PSUM accumulation; den/att/softmax
elementwise kept fp32.
"""
import os
import numpy as np
import ml_dtypes

import concourse.bacc as bacc
import concourse.tile as tile
from concourse import mybir
from concourse.bass_utils import run_bass_kernel_spmd

F32 = mybir.dt.float32
F32R = mybir.dt.float32r
BF16 = mybir.dt.bfloat16
EXP = mybir.ActivationFunctionType.Exp
LN = mybir.ActivationFunctionType.Ln

S, B, D = 2048, 2, 1024
H, DH, C = 16, 64, 16
HPC = 4            # heads per core
HD = HPC * DH      # 256 model dims per core
T = 128            # chunk
NCH = S // T       # 16 chunks
KT = D // 128      # 8 k-tiles of the model dim

LAST_EXEC_NS = None
_CACHE = {}


def _r(ap):
    return ap.bitcast(F32R)


def _grp(ap2d):
    """(128, ...) -> (4, 16, ...): the valid 16 c-rows of each 32-group."""
    return ap2d.rearrange("(g r) x -> g r x", g=4)[:, 0:C, :]


def _patched_insert_act_table_loads(self):
    """Force every activation (Exp/Ln/Copy/Identity) onto the single
    act-func table that contains them all. The default greedy pass picks
    exp_and_others for Exp and natural_log_exp_and_others for Ln, inserting
    an ACT_TABLE_LOAD (1.28us!) on every switch -- 65 loads = 83us."""
    import types as _t  # noqa: F401
    import bass_rust as _bass_rust
    from concourse.hw_specs import get_activation_tables
    has_activation = any(
        isinstance(i, mybir.InstActivation)
        for b in self.main_func.blocks
        for i in b.instructions)
    if not has_activation:
        return
    keep = {mybir.ActivationFunctionType.Exp, mybir.ActivationFunctionType.Ln,
            mybir.ActivationFunctionType.Copy,
            mybir.ActivationFunctionType.Identity}
    tables = []
    for name, funcs in get_activation_tables(self.m.arch).items():
        if name != "natural_log_exp_and_others":
            funcs = funcs - keep
        tables.append((name, funcs))
    _bass_rust.insert_act_table_loads(self, tables)


def _build():
    nc = bacc.Bacc("TRN2", target_bir_lowering=False, debug=False, num_devices=8)
    import types as _types
    nc.insert_act_table_loads = _types.MethodType(
        _patched_insert_act_table_loads, nc)

    # ---- DRAM I/O (per-core, host-prepped layouts) ----
    xT_d = nc.dram_tensor("xT", [D, S], F32R, kind="ExternalInput")
    wqT_d = nc.dram_tensor("wqT", [D, HD], F32R, kind="ExternalInput")
    wkT_d = nc.dram_tensor("wkT", [D, HD], F32R, kind="ExternalInput")
    wvT_d = nc.dram_tensor("wvT", [D, HD], F32R, kind="ExternalInput")
    woT_d = nc.dram_tensor("woT", [HD, D], F32, kind="ExternalInput")
    qcT_d = nc.dram_tensor("qcT", [64, HPC, C], F32, kind="ExternalInput")
    bq_d = nc.dram_tensor("bq", [HD], F32, kind="ExternalInput")
    bk_d = nc.dram_tensor("bk", [HD], F32, kind="ExternalInput")
    bv_d = nc.dram_tensor("bv", [HD], F32, kind="ExternalInput")
    bo_d = nc.dram_tensor("bo", [D], F32, kind="ExternalInput")
    out_d = nc.dram_tensor("out_p", [S, D], F32, kind="ExternalOutput")

    # ---- consts baked into the NEFF ----
    u = np.triu(np.ones((T, T), np.float32))
    u4 = np.tile(u, (1, 4)).astype(ml_dtypes.bfloat16)
    u4_d = nc.inline_tensor(u4, "u4c")
    ones16_d = nc.inline_tensor(np.ones((C, 1), ml_dtypes.bfloat16), "ones16c")
    ones164_d = nc.inline_tensor(np.ones((1, 64), np.float32), "ones164c")
    zcol_d = nc.inline_tensor(np.zeros((128, 1), np.float32), "zcolc")
    kv_scratch = [nc.dram_tensor(f"kvs{j}", [128, 2048], mybir.dt.bfloat16)
                  for j in range(4)]

    with tile.TileContext(nc) as tc:
        _emit(nc, tc, locals())
    nc.compile()
    return nc


def _emit(nc, tc, d):
    from contextlib import ExitStack
    import concourse.bass as bass

    with ExitStack() as ctx:
        ep = ctx.enter_context

        # ---------- persistent pools ----------
        consts = ep(tc.tile_pool(name="consts", bufs=1))
        wpool = ep(tc.tile_pool(name="wpool", bufs=1))      # WoT + biases
        qkv = ep(tc.tile_pool(name="qkv", bufs=1))          # qT/kT/vT/knat/vnat
        wstore = ep(tc.tile_pool(name="wstore", bufs=1))    # w16, oT
        sb2 = ep(tc.tile_pool(name="sb2", bufs=2))          # per-chunk sbuf
        sb2b = ep(tc.tile_pool(name="sb2b", bufs=2))        # masked G / PW
        snapp = ep(tc.tile_pool(name="snapp", bufs=2))      # SK/SV snapshots
        outp = ep(tc.tile_pool(name="outp", bufs=3))        # out staging

        pp_big = ep(tc.tile_pool(name="pp_big", bufs=2, space="PSUM"))
        pp_ba = ep(tc.tile_pool(name="pp_ba", bufs=1, space="PSUM"))
        pp_misc = ep(tc.tile_pool(name="pp_misc", bufs=2, space="PSUM"))
        pp_tp = ep(tc.tile_pool(name="pp_tp", bufs=1, space="PSUM"))
        pp_sum = ep(tc.tile_pool(name="pp_sum", bufs=1, space="PSUM"))

        # ---------- consts ----------
        u4_t = consts.tile([128, 4 * T], BF16, tag="u4")
        nc.sync.dma_start(out=u4_t, in_=d["u4_d"].ap())
        u128 = u4_t[:, 0:T]
        ones16_t = consts.tile([C, 1], BF16, tag="ones16")
        nc.sync.dma_start(out=ones16_t, in_=d["ones16_d"].ap())
        ones164_t = consts.tile([1, 64], F32, tag="ones164")
        nc.sync.dma_start(out=ones164_t, in_=d["ones164_d"].ap())
        zcol_t = consts.tile([128, 1], F32, tag="zcol")
        nc.sync.dma_start(out=zcol_t, in_=d["zcol_d"].ap())

        qcT_t = consts.tile([64, HPC, C], BF16, tag="qcT")
        nc.gpsimd.dma_start(out=qcT_t, in_=d["qcT_d"].ap())  # casting dma

        # biases as (128, 2) per-partition columns
        def bias_tile(name):
            t = consts.tile([128, 2], F32, tag=name)
            nc.sync.dma_start(
                out=t, in_=d[name + "_d"].ap().rearrange("(m p) -> p m", p=128))
            return t
        bq_t, bk_t, bv_t = bias_tile("bq"), bias_tile("bk"), bias_tile("bv")

        # ---------- projections ----------
        qT = [qkv.tile([128, S], BF16, tag=f"qT{m}", name=f"qT{m}") for m in range(2)]
        kT = [qkv.tile([128, S], BF16, tag=f"kT{m}", name=f"kT{m}") for m in range(2)]
        vT = [qkv.tile([128, S], BF16, tag=f"vT{m}", name=f"vT{m}") for m in range(2)]

        with tc.tile_pool(name="xw", bufs=1) as xw:
            xT_t = xw.tile([128, KT, S], F32R, tag="xT")
            xr = d["xT_d"].ap().rearrange("(kt p) s -> kt p s", p=128)
            for kt in range(KT):
                nc.sync.dma_start(out=xT_t[:, kt, :], in_=xr[kt])
            wts = {}
            for nm in ("wq", "wk", "wv"):
                wt = xw.tile([128, KT, HD], F32R, tag=nm)
                wr = d[nm + "T_d"].ap().rearrange("(kt p) j -> kt p j", p=128)
                for kt in range(KT):
                    nc.sync.dma_start(out=wt[:, kt, :], in_=wr[kt])
                wts[nm] = wt

            # psum -> sbuf epilogue: copy + per-partition bias; alternate
            # DVE / Act so neither engine owns all 24 copies. (The q scale
            # 0.125*exp(-beta) is folded into wq/bq on the host.)
            epi = 0
            for nm, dst, bias in (
                    ("wq", qT, bq_t), ("wk", kT, bk_t), ("wv", vT, bv_t)):
                for m in range(2):
                    for n in range(4):
                        ps = pp_big.tile([128, 512], F32, tag="big")
                        for kt in range(KT):
                            nc.tensor.matmul(
                                ps, wts[nm][:, kt, 128 * m:128 * m + 128],
                                xT_t[:, kt, 512 * n:512 * n + 512],
                                start=(kt == 0), stop=(kt == KT - 1))
                        if epi % 2 == 0:
                            nc.vector.tensor_scalar(
                                out=dst[m][:, 512 * n:512 * n + 512], in0=ps,
                                scalar1=bias[:, m:m + 1], scalar2=None,
                                op0=mybir.AluOpType.add)
                        else:
                            nc.scalar.activation(
                                out=dst[m][:, 512 * n:512 * n + 512], in_=ps,
                                func=mybir.ActivationFunctionType.Identity,
                                bias=bias[:, m:m + 1])
                        epi += 1

        # post-projection tiles (allocated after the x/W pool is released)
        qkv2 = ep(tc.tile_pool(name="qkv2", bufs=1))
        knat = [qkv2.tile([128, NCH, 128], BF16, tag=f"knat{m}", name=f"knat{m}")
                for m in range(2)]
        vnat = [qkv2.tile([128, NCH, 128], BF16, tag=f"vnat{m}", name=f"vnat{m}")
                for m in range(2)]

        # k/v natural layout via DMA xbar transpose (bf16), staged via DRAM
        # (SBUF-source xbar transpose is fatal on HW)
        kvs = d["kv_scratch"]
        for m in range(2):
            nc.sync.dma_start(out=kvs[m].ap(), in_=kT[m][:, :])
            nc.sync.dma_start_transpose(out=knat[m], in_=kvs[m].ap())
            nc.sync.dma_start(out=kvs[2 + m].ap(), in_=vT[m][:, :])
            nc.sync.dma_start_transpose(out=vnat[m], in_=kvs[2 + m].ap())

        # row-0 copies of odd heads' q/k (PE row-position changes between
        # back-to-back matmuls are fatal on HW; keep every operand at row 0)
        qTo = [qkv2.tile([64, S], BF16, tag=f"qTo{j}", name=f"qTo{j}") for j in range(2)]
        kTo = [qkv2.tile([64, S], BF16, tag=f"kTo{j}", name=f"kTo{j}") for j in range(2)]
        for j in range(2):
            nc.sync.dma_start(out=qTo[j], in_=qT[j][64:128, :])
            nc.sync.dma_start(out=kTo[j], in_=kT[j][64:128, :])

        def qTh(h):
            return qT[h // 2][0:64, :] if h % 2 == 0 else qTo[h // 2][:, :]

        def kTh(h):
            return kT[h // 2][0:64, :] if h % 2 == 0 else kTo[h // 2][:, :]

        # WoT resident (bf16): (128, 2, D); bo broadcast tile (128, D) * 0.25
        woT_t = wpool.tile([128, 2, D], BF16, tag="woT")
        wor = d["woT_d"].ap().rearrange("(kt p) j -> kt p j", p=128)
        for kt in range(2):
            nc.gpsimd.dma_start(out=woT_t[:, kt, :], in_=wor[kt])
        bo_b = wpool.tile([128, D], F32, tag="bo_b")
        nc.sync.dma_start(out=bo_b, in_=bass.AP(
            tensor=d["bo_d"].ap().tensor, offset=0, ap=[[0, 128], [1, D]]))
        nc.scalar.mul(out=bo_b, in_=bo_b, mul=0.25)

        # ---------- stage 1: w16 = exp(logits), free-packed (c, h, s) ----------
        w16 = wstore.tile([C, HPC, S], BF16, tag="w16")
        for n in range(4):
            ps = pp_big.tile([128, 512], F32, tag="big")
            for h in range(HPC):
                nc.tensor.matmul(
                    ps[32 * h:32 * h + C, :], qcT_t[:, h, :],
                    kTh(h)[:, 512 * n:512 * n + 512],
                    start=True, stop=True, tile_position=(0, 32 * h))
            for h in range(HPC):
                nc.scalar.activation(
                    out=w16[:, h, 512 * n:512 * n + 512],
                    in_=ps[32 * h:32 * h + C, :], func=EXP)

        # ---------- stage 2: chunks ----------
        oT = [wstore.tile([128, S], BF16, tag=f"oT{m}", name=f"oT{m}") for m in range(2)]
        sk_sb = None   # (64, HPC, C) bf16 [d, h, c]
        sv_sb = None   # (C, HPC, DH) bf16 [c, h, d]
        den_prev = None

        for i in range(NCH):
            ch = slice(T * i, T * i + T)

            # wT (t, c4-packed) = exp(k . qc) via transposed-logits matmuls
            tp = pp_tp.tile([128, 4 * C], F32, tag="tp")
            for h in range(HPC):
                nc.tensor.matmul(
                    tp[:, C * h:C * h + C], kTh(h)[:, ch], qcT_t[:, h, :],
                    start=True, stop=True)
            wT_sb = sb2.tile([128, 4 * C], BF16, tag="wT")
            nc.scalar.activation(out=wT_sb, in_=tp, func=EXP)

            # G (t, s) for 4 heads + causal mask
            gt = pp_big.tile([128, 512], F32, tag="big")
            for h in range(HPC):
                nc.tensor.matmul(
                    gt[:, 128 * h:128 * h + 128],
                    kTh(h)[:, ch], qTh(h)[:, ch], start=True, stop=True)
            gmt = sb2b.tile([128, 512], BF16, tag="gmt")
            nc.vector.tensor_mul(gmt, gt, u4_t)

            ba = pp_ba.tile([C, 2, HPC, T], F32, tag="ba")
            den_ps = ba[:, 0]           # (C, HPC, T)
            an_ps = ba[:, 1]
            misc = pp_misc.tile([64, 512], F32, tag="misc")
            sv_ps = misc[0:C, 0:256].rearrange("c (h v) -> c h v", h=HPC)
            sk_ps = misc[:, 256:320].rearrange("p (h c) -> p h c", h=HPC)

            # den / att_num / state deltas (all row-0 matmuls)
            for h in range(HPC):
                nc.tensor.matmul(den_ps[:, h, :], wT_sb[:, C * h:C * h + C],
                                 u128, start=True, stop=True)
                nc.tensor.matmul(
                    an_ps[:, h, :], wT_sb[:, C * h:C * h + C],
                    gmt[:, 128 * h:128 * h + 128],
                    start=True, stop=(sk_sb is None))
                if sk_sb is not None:
                    nc.tensor.matmul(an_ps[:, h, :], sk_sb[:, h, :],
                                     qTh(h)[:, ch], start=False, stop=True)
                nc.tensor.matmul(
                    sv_ps[:, h, :], wT_sb[:, C * h:C * h + C],
                    vnat[h // 2][:, i, 64 * (h % 2):64 * (h % 2) + 64],
                    start=True, stop=True)
                nc.tensor.matmul(
                    sk_ps[:, h, :],
                    knat[h // 2][:, i, 64 * (h % 2):64 * (h % 2) + 64],
                    wT_sb[:, C * h:C * h + C], start=True, stop=True)

            den_sb = sb2.tile([C, HPC, T], F32, tag="den")
            for h in range(HPC):
                carry = (zcol_t[0:C, :] if den_prev is None
                         else den_prev[:, h, T - 1:T])
                nc.vector.tensor_scalar(
                    out=den_sb[:, h, :], in0=den_ps[:, h, :], scalar1=carry,
                    scalar2=None, op0=mybir.AluOpType.add)
            den_prev = den_sb
            # 1/x as exp(-ln(x)) on the Act engine: DVE reciprocal costs
            # ~6.4 cyc/elem (3.3us per 512-free op); ln+exp share one act
            # table with the other Exp calls, so no table reloads.
            lden = sb2.tile([C, HPC, T], F32, tag="lden")
            nc.scalar.activation(out=lden, in_=den_sb, func=LN)
            rden = sb2.tile([C, HPC, T], F32, tag="rden")
            nc.scalar.activation(out=rden, in_=lden, func=EXP, scale=-1.0)

            att = sb2.tile([C, HPC, T], F32, tag="att")
            nc.vector.tensor_mul(att, an_ps, rden)
            e_sb = sb2.tile([C, HPC, T], BF16, tag="e")
            nc.scalar.activation(out=e_sb, in_=att, func=EXP)
            sums_ps = pp_sum.tile([1, HPC, T], F32, tag="sums")
            nc.tensor.matmul(sums_ps[:, :, :].rearrange("o h t -> o (h t)"),
                             ones16_t,
                             e_sb.rearrange("c h t -> c (h t)"),
                             start=True, stop=True)
            lsums = sb2.tile([1, HPC, T], F32, tag="lsums")
            nc.scalar.activation(out=lsums, in_=sums_ps, func=LN)
            rsums = sb2.tile([1, HPC, T], F32, tag="rsums")
            nc.scalar.activation(out=rsums, in_=lsums, func=EXP, scale=-1.0)
            pdd = sb2.tile([C, HPC, T], BF16, tag="pdd")
            nc.vector.tensor_mul(pdd, e_sb, rden)

            # PW (t, s) + mask
            pw = pp_big.tile([128, 512], F32, tag="big")
            for h in range(HPC):
                nc.tensor.matmul(
                    pw[:, 128 * h:128 * h + 128], w16[:, h, ch],
                    pdd[:, h, :], start=True, stop=True)
            pwm = sb2b.tile([128, 512], BF16, tag="pwm")
            nc.vector.tensor_mul(pwm, pw, u4_t)

            # oT pair tiles + rsum broadcast (cols 256-511)
            mix2 = pp_big.tile([128, 512], F32, tag="big")
            for h in range(HPC):
                bp = 64 * (h % 2)
                nc.tensor.matmul(
                    mix2[bp:bp + 64, 128 * (h // 2):128 * (h // 2) + 128],
                    vnat[h // 2][:, i, bp:bp + 64],
                    pwm[:, 128 * h:128 * h + 128],
                    start=True, stop=(sv_sb is None), tile_position=(0, bp))
                if sv_sb is not None:
                    nc.tensor.matmul(
                        mix2[bp:bp + 64, 128 * (h // 2):128 * (h // 2) + 128],
                        sv_sb[:, h, :], pdd[:, h, :],
                        start=False, stop=True, tile_position=(0, bp))
                nc.tensor.matmul(
                    mix2[bp:bp + 64, 256 + 128 * (h // 2):384 + 128 * (h // 2)],
                    ones164_t, rsums[:, h, :],
                    start=True, stop=True, tile_position=(0, bp))
            rb_sb = sb2.tile([128, 256], F32, tag="rb")
            nc.scalar.copy(out=rb_sb, in_=mix2[:, 256:512])
            for m in range(2):
                nc.vector.tensor_mul(oT[m][:, ch], mix2[:, 128 * m:128 * m + 128],
                                     rb_sb[:, 128 * m:128 * m + 128])

            # state accumulation into sbuf
            new_sk = snapp.tile([64, HPC, C], BF16, tag="sk")
            new_sv = snapp.tile([C, HPC, DH], BF16, tag="sv")
            if sk_sb is None:
                nc.scalar.copy(out=new_sk, in_=sk_ps)
                nc.scalar.copy(out=new_sv, in_=sv_ps)
            else:
                nc.vector.tensor_add(new_sk, sk_ps, sk_sb)
                nc.vector.tensor_add(new_sv, sv_ps, sv_sb)
            sk_sb, sv_sb = new_sk, new_sv

        # ---------- out-projection ----------
        for sc in range(NCH):
            ch = slice(T * sc, T * sc + T)
            ob = outp.tile([128, D], F32, tag="ob")
            for n2 in range(2):
                ps = pp_big.tile([128, 512], F32, tag="big")
                for kt in range(2):
                    nc.tensor.matmul(
                        ps, oT[kt][:, ch],
                        woT_t[:, kt, 512 * n2:512 * n2 + 512],
                        start=(kt == 0), stop=(kt == 1))
                nc.vector.tensor_add(ob[:, 512 * n2:512 * n2 + 512], ps,
                                     bo_b[:, 512 * n2:512 * n2 + 512])
            nc.sync.dma_start(out=d["out_d"].ap()[ch, :], in_=ob)


def kernel(**inputs):
    global LAST_EXEC_NS
    x = np.ascontiguousarray(inputs["x"], np.float32)
    q_c, beta = np.asarray(inputs["q_c"]), np.asarray(inputs["beta"])
    Wq, bq = np.asarray(inputs["Wq"]), np.asarray(inputs["bq"])
    Wk, bk = np.asarray(inputs["Wk"]), np.asarray(inputs["bk"])
    Wv, bv = np.asarray(inputs["Wv"]), np.asarray(inputs["bv"])
    Wo, bo = np.asarray(inputs["Wo"]), np.asarray(inputs["bo"])

    if "nc" not in _CACHE:
        _CACHE["nc"] = _build()
    nc = _CACHE["nc"]

    # per-head query temperature 0.125*exp(-beta) folded into Wq/bq
    qscale = (0.125 * np.exp(-beta.astype(np.float64))).astype(np.float32)
    qs_hd = np.repeat(qscale, DH)                      # (D,) per out-dim
    Wq_s = Wq * qs_hd[:, None]
    bq_s = bq * qs_hd

    in_maps = []
    for core in range(8):
        b, hh = core // 4, core % 4
        hd = slice(hh * HD, hh * HD + HD)
        qct = np.zeros((64, HPC, C), np.float32)
        qc_r = q_c[:, hd].reshape(C, HPC, DH)          # (c, h, d)
        for h in range(HPC):
            qct[:, h, :] = qc_r[:, h, :].T
        in_maps.append({
            "xT": np.ascontiguousarray(x[:, b, :].T),
            "wqT": np.ascontiguousarray(Wq_s[hd, :].T),
            "wkT": np.ascontiguousarray(Wk[hd, :].T),
            "wvT": np.ascontiguousarray(Wv[hd, :].T),
            "woT": np.ascontiguousarray(Wo[:, hd].T),
            "qcT": qct,
            "bq": np.ascontiguousarray(bq_s[hd]),
            "bk": np.ascontiguousarray(bk[hd]),
            "bv": np.ascontiguousarray(bv[hd]),
            "bo": np.ascontiguousarray(bo),
        })

    trace = os.environ.get("TRN_PROFILE", "0") == "1"
    res = run_bass_kernel_spmd(nc, in_maps, list(range(8)), trace=trace)
    LAST_EXEC_NS = res.exec_time_ns

    out = np.zeros((S, B, D), np.float32)
    for core in range(8):
        out[:, core // 4, :] += res.results[core]["out_p"]
    return out



# revision 16
# speedup vs baseline: 1.3888x; 1.3888x over previous
"""CompressionAttention Trainium2 kernel (8 NeuronCores, SPMD).

Sharding: core i handles batch b=i//4 and 4 heads hh=i%4 (model-dim slice
hh*256:(hh+1)*256). Heads never interact before out_proj, so each core
computes a partial out-projection for its batch; the host gather sums the
4 partials per batch (tensor-parallel unshard) -- bo is added on-device
as bo/4 by each core.

Algorithm per core (chunked linear attention, chunk T=128):
  w[c,t] = exp(qc_c . k_t)            (max-subtraction dropped: att is
                                       invariant to per-c scaling of w)
  den[c,s]   = cumsum_t<=s w[c,t]
  att[c,s]   = (w_chunk^T (U o G) + SK^T qs) / den   per chunk + state
  p = softmax_c att ; o_s = sum_c p * (sum_t<=s w v)/den

Schedule: all cross-chunk recurrences (den carry, SK/SV prefixes) are
precomputed in a cheap side-phase (tiny matmuls + DVE adds), so the main
chunk loop has no chunk(i) -> chunk(i+1) dependency and pipelines freely
across engines. Softmax 1/x runs as exp(-ln(x)) on the Act engine (single
act table); the 1/sums normalization is deferred out of the loop entirely
(sums broadcast via ones-matmul, one big multiply at the end).
"""
import os
import numpy as np
import ml_dtypes

import concourse.bacc as bacc
import concourse.tile as tile
from concourse import mybir
from concourse.bass_utils import run_bass_kernel_spmd

F32 = mybir.dt.float32
BF16 = mybir.dt.bfloat16
EXP = mybir.ActivationFunctionType.Exp
LN = mybir.ActivationFunctionType.Ln
IDENT = mybir.ActivationFunctionType.Identity

S, B, D = 2048, 2, 1024
H, DH, C = 16, 64, 16
HPC = 4            # heads per core
HD = HPC * DH      # 256 model dims per core
T = 128            # chunk
NCH = S // T       # 16 chunks
KT = D // 128      # 8 k-tiles of the model dim

LAST_EXEC_NS = None
_CACHE = {}


def _patched_insert_act_table_loads(self):
    """Force every activation (Exp/Ln/Copy/Identity) onto the single
    act-func table that contains them all. The default greedy pass picks
    exp_and_others for Exp and natural_log_exp_and_others for Ln, inserting
    an ACT_TABLE_LOAD (1.28us!) on every switch."""
    import bass_rust as _bass_rust
    from concourse.hw_specs import get_activation_tables
    has_activation = any(
        isinstance(i, mybir.InstActivation)
        for b in self.main_func.blocks
        for i in b.instructions)
    if not has_activation:
        return
    keep = {mybir.ActivationFunctionType.Exp, mybir.ActivationFunctionType.Ln,
            mybir.ActivationFunctionType.Copy,
            mybir.ActivationFunctionType.Identity}
    tables = []
    for name, funcs in get_activation_tables(self.m.arch).items():
        if name != "natural_log_exp_and_others":
            funcs = funcs - keep
        tables.append((name, funcs))
    _bass_rust.insert_act_table_loads(self, tables)


def _build():
    nc = bacc.Bacc("TRN2", target_bir_lowering=False, debug=False, num_devices=8)
    import types as _types
    nc.insert_act_table_loads = _types.MethodType(
        _patched_insert_act_table_loads, nc)

    # ---- DRAM I/O (per-core, host-prepped layouts; bf16 where possible) ----
    xT_d = nc.dram_tensor("xT", [D, S], BF16, kind="ExternalInput")
    wqT_d = nc.dram_tensor("wqT", [D, HD], BF16, kind="ExternalInput")
    wkT_d = nc.dram_tensor("wkT", [D, HD], BF16, kind="ExternalInput")
    wvT_d = nc.dram_tensor("wvT", [D, HD], BF16, kind="ExternalInput")
    woT_d = nc.dram_tensor("woT", [HD, D], BF16, kind="ExternalInput")
    qcT_d = nc.dram_tensor("qcT", [64, HPC, C], BF16, kind="ExternalInput")
    bq_d = nc.dram_tensor("bq", [HD], F32, kind="ExternalInput")
    bk_d = nc.dram_tensor("bk", [HD], F32, kind="ExternalInput")
    bv_d = nc.dram_tensor("bv", [1, HD], BF16, kind="ExternalInput")
    bo4_d = nc.dram_tensor("bo4", [1, D], BF16, kind="ExternalInput")
    out_d = nc.dram_tensor("out_p", [S, D], F32, kind="ExternalOutput")

    # ---- consts baked into the NEFF ----
    u = np.triu(np.ones((T, T), np.float32))
    u4 = np.tile(u, (1, 4)).astype(ml_dtypes.bfloat16)
    u4_d = nc.inline_tensor(u4, "u4c")
    onesc64_d = nc.inline_tensor(np.ones((C, 64), ml_dtypes.bfloat16), "onesc64")
    onescol_d = nc.inline_tensor(np.ones((128, 1), ml_dtypes.bfloat16), "onescol")
    onesrow_d = nc.inline_tensor(np.ones((1, 128), ml_dtypes.bfloat16), "onesrow")
    kv_scratch = [nc.dram_tensor(f"kvs{j}", [128, 2048], BF16) for j in range(2)]

    with tile.TileContext(nc) as tc:
        _emit(nc, tc, locals())
    nc.compile()
    return nc


def _emit(nc, tc, d):
    from contextlib import ExitStack
    import concourse.bass as bass

    with ExitStack() as ctx:
        ep = ctx.enter_context

        # ---------- persistent pools ----------
        consts = ep(tc.tile_pool(name="consts", bufs=1))
        wpool = ep(tc.tile_pool(name="wpool", bufs=1))      # WoT
        qkv = ep(tc.tile_pool(name="qkv", bufs=1))          # qT/kT/vnat
        qkv2 = ep(tc.tile_pool(name="qkv2", bufs=1))        # knat, row-0 copies
        wstore = ep(tc.tile_pool(name="wstore", bufs=1))    # w16/wT/carry/sk/sv
        onum = ep(tc.tile_pool(name="onum", bufs=1))        # o_num, rb_all
        sb2 = ep(tc.tile_pool(name="sb2", bufs=2))          # per-chunk sbuf
        sb2b = ep(tc.tile_pool(name="sb2b", bufs=2))        # masked G / PW
        otp = ep(tc.tile_pool(name="otp", bufs=1))          # oT final
        outp = ep(tc.tile_pool(name="outp", bufs=3))        # out staging

        # ---------- consts ----------
        u4_t = consts.tile([128, 4 * T], BF16, tag="u4")
        nc.sync.dma_start(out=u4_t, in_=d["u4_d"].ap())
        u128 = u4_t[:, 0:T]
        onesc64_t = consts.tile([C, 64], BF16, tag="onesc64")
        nc.sync.dma_start(out=onesc64_t, in_=d["onesc64_d"].ap())
        onescol_t = consts.tile([128, 1], BF16, tag="onescol")
        nc.sync.dma_start(out=onescol_t, in_=d["onescol_d"].ap())
        onesrow_t = consts.tile([1, 128], BF16, tag="onesrow")
        nc.sync.dma_start(out=onesrow_t, in_=d["onesrow_d"].ap())
        qcT_t = consts.tile([64, HPC, C], BF16, tag="qcT")
        nc.sync.dma_start(out=qcT_t, in_=d["qcT_d"].ap())
        bv_t = consts.tile([1, HD], BF16, tag="bv")
        nc.sync.dma_start(out=bv_t, in_=d["bv_d"].ap())
        bo4_t = consts.tile([1, D], BF16, tag="bo4")
        nc.sync.dma_start(out=bo4_t, in_=d["bo4_d"].ap())

        # biases as (128, 2) per-partition columns (q/k only; v uses a row)
        def bias_tile(name):
            t = consts.tile([128, 2], F32, tag=name)
            nc.sync.dma_start(
                out=t, in_=d[name + "_d"].ap().rearrange("(m p) -> p m", p=128))
            return t
        bq_t, bk_t = bias_tile("bq"), bias_tile("bk")

        # WoT resident (bf16): (128, 2, D)
        woT_t = wpool.tile([128, 2, D], BF16, tag="woT")
        wor = d["woT_d"].ap().rearrange("(kt p) j -> kt p j", p=128)
        for kt in range(2):
            nc.sync.dma_start(out=woT_t[:, kt, :], in_=wor[kt])

        # ---------- projections (all bf16) ----------
        qT = [qkv.tile([128, S], BF16, tag=f"qT{m}", name=f"qT{m}") for m in range(2)]
        kT = [qkv.tile([128, S], BF16, tag=f"kT{m}", name=f"kT{m}") for m in range(2)]
        vnat = qkv.tile([128, NCH, HD], BF16, tag="vnat", name="vnat")

        with tc.tile_pool(name="xw", bufs=1) as xw, \
                tc.tile_pool(name="ppj", bufs=2, space="PSUM") as ppj:
            xT_t = xw.tile([128, KT, S], BF16, tag="xT")
            xr = d["xT_d"].ap().rearrange("(kt p) s -> kt p s", p=128)
            for kt in range(KT):
                nc.sync.dma_start(out=xT_t[:, kt, :], in_=xr[kt])
            wts = {}
            for nm in ("wq", "wk", "wv"):
                wt = xw.tile([128, KT, HD], BF16, tag=nm)
                wr = d[nm + "T_d"].ap().rearrange("(kt p) j -> kt p j", p=128)
                for kt in range(KT):
                    nc.sync.dma_start(out=wt[:, kt, :], in_=wr[kt])
                wts[nm] = wt

            epi = 0
            for nm, dst, bias in (("wq", qT, bq_t), ("wk", kT, bk_t)):
                for m in range(2):
                    for n in range(4):
                        ps = ppj.tile([128, 512], F32, tag="pj")
                        for kt in range(KT):
                            nc.tensor.matmul(
                                ps, wts[nm][:, kt, 128 * m:128 * m + 128],
                                xT_t[:, kt, 512 * n:512 * n + 512],
                                start=(kt == 0), stop=(kt == KT - 1))
                        if epi % 2 == 0:
                            nc.vector.tensor_scalar(
                                out=dst[m][:, 512 * n:512 * n + 512], in0=ps,
                                scalar1=bias[:, m:m + 1], scalar2=None,
                                op0=mybir.AluOpType.add)
                        else:
                            nc.scalar.activation(
                                out=dst[m][:, 512 * n:512 * n + 512], in_=ps,
                                func=IDENT, bias=bias[:, m:m + 1])
                        epi += 1

            # v directly in natural (s, hd) layout: out = xT.T @ wvT
            for sc in range(NCH):
                vp = ppj.tile([128, HD], F32, tag="vp")
                for kt in range(KT):
                    nc.tensor.matmul(
                        vp, xT_t[:, kt, T * sc:T * sc + T], wts["wv"][:, kt, :],
                        start=(kt == 0), stop=False)
                nc.tensor.matmul(vp, onesrow_t, bv_t, start=False, stop=True)
                if sc % 2 == 0:
                    nc.vector.tensor_copy(out=vnat[:, sc, :], in_=vp)
                else:
                    nc.scalar.copy(out=vnat[:, sc, :], in_=vp)

        # k natural layout via DMA xbar transpose, staged via DRAM
        # (SBUF-source xbar transpose is fatal on HW)
        knat = [qkv2.tile([128, NCH, 128], BF16, tag=f"knat{m}", name=f"knat{m}")
                for m in range(2)]
        kvs = d["kv_scratch"]
        for m in range(2):
            nc.sync.dma_start(out=kvs[m].ap(), in_=kT[m][:, :])
            nc.sync.dma_start_transpose(out=knat[m], in_=kvs[m].ap())

        # row-0 copies of odd heads' q/k (PE row-position changes between
        # back-to-back matmuls are fatal on HW; keep every operand at row 0)
        qTo = [qkv2.tile([64, S], BF16, tag=f"qTo{j}", name=f"qTo{j}") for j in range(2)]
        kTo = [qkv2.tile([64, S], BF16, tag=f"kTo{j}", name=f"kTo{j}") for j in range(2)]
        for j in range(2):
            nc.sync.dma_start(out=qTo[j], in_=qT[j][64:128, :])
            nc.sync.dma_start(out=kTo[j], in_=kT[j][64:128, :])

        def qTh(h):
            return qT[h // 2][0:64, :] if h % 2 == 0 else qTo[h // 2][:, :]

        def kTh(h):
            return kT[h // 2][0:64, :] if h % 2 == 0 else kTo[h // 2][:, :]

        # ---------- side phase: w16, wT, den carries, SK/SV prefixes ----------
        w16 = wstore.tile([C, HPC, S], BF16, tag="w16")
        wT_all = wstore.tile([128, NCH, 4 * C], BF16, tag="wT")
        carry_f = wstore.tile([1, NCH, 4 * C], F32, tag="carry_f")
        carry_bf = wstore.tile([1, NCH, 4 * C], BF16, tag="carry_bf")
        sk_all = wstore.tile([64, NCH, HPC, C], BF16, tag="sk")
        sv_all = wstore.tile([C, NCH, HPC, DH], BF16, tag="sv")

        with tc.tile_pool(name="ph3a", bufs=2, space="PSUM") as ph3a:
            # w16 = exp(qc . k), free-packed (c, h, s)
            for n in range(4):
                ps = ph3a.tile([128, 512], F32, tag="w16ps")
                for h in range(HPC):
                    nc.tensor.matmul(
                        ps[32 * h:32 * h + C, :], qcT_t[:, h, :],
                        kTh(h)[:, 512 * n:512 * n + 512],
                        start=True, stop=True, tile_position=(0, 32 * h))
                for h in range(HPC):
                    nc.scalar.activation(
                        out=w16[:, h, 512 * n:512 * n + 512],
                        in_=ps[32 * h:32 * h + C, :], func=EXP)

        with tc.tile_pool(name="ph3b", bufs=2, space="PSUM") as ph3b:
            # wT (t, c4-packed) = exp(k . qc); then cs -> den carry chain
            cs_prev = None
            for i in range(NCH):
                ch = slice(T * i, T * i + T)
                tp = ph3b.tile([128, 4 * C], F32, tag="tp")
                for h in range(HPC):
                    nc.tensor.matmul(
                        tp[:, C * h:C * h + C], kTh(h)[:, ch], qcT_t[:, h, :],
                        start=True, stop=True)
                nc.scalar.activation(out=wT_all[:, i, :], in_=tp, func=EXP)
                if i < NCH - 1:
                    cs = ph3b.tile([1, 4 * C], F32, tag="cs")
                    nc.tensor.matmul(cs, onescol_t, wT_all[:, i, :],
                                     start=True, stop=True)
                    if cs_prev is None:
                        nc.scalar.copy(out=carry_f[:, 1, :], in_=cs)
                    else:
                        nc.vector.tensor_add(carry_f[:, i + 1, :], cs,
                                             carry_f[:, i, :])
                    nc.scalar.copy(out=carry_bf[:, i + 1, :],
                                   in_=carry_f[:, i + 1, :])
                    cs_prev = cs

        with tc.tile_pool(name="ph3c", bufs=2, space="PSUM") as ph3c:
            # SK/SV chunk deltas + exclusive prefixes (slot i = state
            # before chunk i; slot 0 unused)
            skd_prev = svd_prev = None
            for i in range(NCH - 1):
                skd = ph3c.tile([64, HPC, C], F32, tag="skd")
                svd = ph3c.tile([C, HPC, DH], F32, tag="svd")
                for h in range(HPC):
                    nc.tensor.matmul(
                        skd[:, h, :],
                        knat[h // 2][:, i, 64 * (h % 2):64 * (h % 2) + 64],
                        wT_all[:, i, C * h:C * h + C], start=True, stop=True)
                    nc.tensor.matmul(
                        svd[:, h, :], wT_all[:, i, C * h:C * h + C],
                        vnat[:, i, 64 * h:64 * h + 64], start=True, stop=True)
                if skd_prev is None:
                    nc.scalar.copy(out=sk_all[:, 1], in_=skd)
                    nc.scalar.copy(out=sv_all[:, 1], in_=svd)
                else:
                    nc.vector.tensor_add(sk_all[:, i + 1], skd, sk_all[:, i])
                    nc.vector.tensor_add(sv_all[:, i + 1], svd, sv_all[:, i])
                skd_prev, svd_prev = skd, svd

        # ---------- main chunk loop (no cross-chunk dependencies) ----------
        o_nm = onum.tile([128, 2, S], BF16, tag="o_nm")
        rb_all = onum.tile([128, 2, S], BF16, tag="rb_all")

        pgb = ep(tc.tile_pool(name="pgb", bufs=2, space="PSUM"))
        pden = ep(tc.tile_pool(name="pden", bufs=2, space="PSUM"))
        pan = ep(tc.tile_pool(name="pan", bufs=2, space="PSUM"))
        pmr = ep(tc.tile_pool(name="pmr", bufs=2, space="PSUM"))

        for i in range(NCH):
            ch = slice(T * i, T * i + T)

            # G (t, s) for 4 heads + causal mask
            gt = pgb.tile([128, 512], F32, tag="big")
            for h in range(HPC):
                nc.tensor.matmul(
                    gt[:, 128 * h:128 * h + 128],
                    kTh(h)[:, ch], qTh(h)[:, ch], start=True, stop=True)
            gmt = sb2b.tile([128, 512], BF16, tag="gmt")
            nc.vector.tensor_mul(gmt, gt, u4_t)

            # den / att numerator, carries and state via matmul accumulation
            den_ps = pden.tile([C, HPC, T], F32, tag="den")
            an_ps = pan.tile([C, HPC, T], F32, tag="an")
            for h in range(HPC):
                nc.tensor.matmul(den_ps[:, h, :], wT_all[:, i, C * h:C * h + C],
                                 u128, start=True, stop=(i == 0))
                if i > 0:
                    nc.tensor.matmul(den_ps[:, h, :],
                                     carry_bf[:, i, C * h:C * h + C],
                                     onesrow_t, start=False, stop=True)
                nc.tensor.matmul(
                    an_ps[:, h, :], wT_all[:, i, C * h:C * h + C],
                    gmt[:, 128 * h:128 * h + 128],
                    start=True, stop=(i == 0))
                if i > 0:
                    nc.tensor.matmul(an_ps[:, h, :], sk_all[:, i, h, :],
                                     qTh(h)[:, ch], start=False, stop=True)

            # softmax pieces: 1/x as exp(-ln x) on Act
            lden = sb2.tile([C, HPC, T], F32, tag="lden")
            nc.scalar.activation(out=lden, in_=den_ps, func=LN)
            rden = sb2.tile([C, HPC, T], F32, tag="rden")
            nc.scalar.activation(out=rden, in_=lden, func=EXP, scale=-1.0)
            att = sb2.tile([C, HPC, T], F32, tag="att")
            nc.vector.tensor_mul(att, an_ps, rden)
            e_sb = sb2.tile([C, HPC, T], BF16, tag="e")
            nc.scalar.activation(out=e_sb, in_=att, func=EXP)
            # pdd on GpSimd: SBUF-only operands, keeps DVE free
            pdd = sb2.tile([C, HPC, T], BF16, tag="pdd")
            nc.gpsimd.tensor_mul(pdd, e_sb, rden)

            # PW (t, s) + mask
            pw = pgb.tile([128, 512], F32, tag="big")
            for h in range(HPC):
                nc.tensor.matmul(
                    pw[:, 128 * h:128 * h + 128], w16[:, h, ch],
                    pdd[:, h, :], start=True, stop=True)
            pwm = sb2b.tile([128, 512], BF16, tag="pwm")
            nc.vector.tensor_mul(pwm, pw, u4_t)

            # o numerator (cols 0-255) + sums broadcast (cols 256-511)
            mr = pmr.tile([128, 512], F32, tag="mr")
            for h in range(HPC):
                bp = 64 * (h % 2)
                cb = 128 * (h // 2)
                nc.tensor.matmul(
                    mr[bp:bp + 64, cb:cb + 128],
                    vnat[:, i, 64 * h:64 * h + 64],
                    pwm[:, 128 * h:128 * h + 128],
                    start=True, stop=(i == 0), tile_position=(0, bp))
                if i > 0:
                    nc.tensor.matmul(
                        mr[bp:bp + 64, cb:cb + 128],
                        sv_all[:, i, h, :], pdd[:, h, :],
                        start=False, stop=True, tile_position=(0, bp))
                nc.tensor.matmul(
                    mr[bp:bp + 64, 256 + cb:256 + cb + 128],
                    onesc64_t, e_sb[:, h, :],
                    start=True, stop=True, tile_position=(0, bp))
            nc.vector.tensor_copy(out=o_nm[:, :, ch], in_=mr[:, 0:256])
            lsb = sb2.tile([128, 256], F32, tag="lsb")
            nc.scalar.activation(out=lsb, in_=mr[:, 256:512], func=LN)
            nc.scalar.activation(out=rb_all[:, :, ch], in_=lsb,
                                 func=EXP, scale=-1.0)

        # ---------- tail: deferred softmax normalization + out-proj ----------
        oT = [otp.tile([128, S], BF16, tag=f"oT{m}", name=f"oT{m}")
              for m in range(2)]
        for m in range(2):
            nc.vector.tensor_mul(oT[m], o_nm[:, m, :], rb_all[:, m, :])

        for sc in range(NCH):
            ch = slice(T * sc, T * sc + T)
            ob = outp.tile([128, D], F32, tag="ob")
            for n2 in range(2):
                ps = pgb.tile([128, 512], F32, tag="big")
                for kt in range(2):
                    nc.tensor.matmul(
                        ps, oT[kt][:, ch],
                        woT_t[:, kt, 512 * n2:512 * n2 + 512],
                        start=(kt == 0), stop=False)
                nc.tensor.matmul(
                    ps, onesrow_t, bo4_t[:, 512 * n2:512 * n2 + 512],
                    start=False, stop=True)
                if n2 == 0:
                    nc.vector.tensor_copy(
                        out=ob[:, 512 * n2:512 * n2 + 512], in_=ps)
                else:
                    nc.scalar.copy(
                        out=ob[:, 512 * n2:512 * n2 + 512], in_=ps)
            nc.sync.dma_start(out=d["out_d"].ap()[ch, :], in_=ob)


def kernel(**inputs):
    global LAST_EXEC_NS
    x = np.ascontiguousarray(inputs["x"], np.float32)
    q_c, beta = np.asarray(inputs["q_c"]), np.asarray(inputs["beta"])
    Wq, bq = np.asarray(inputs["Wq"]), np.asarray(inputs["bq"])
    Wk, bk = np.asarray(inputs["Wk"]), np.asarray(inputs["bk"])
    Wv, bv = np.asarray(inputs["Wv"]), np.asarray(inputs["bv"])
    Wo, bo = np.asarray(inputs["Wo"]), np.asarray(inputs["bo"])

    if "nc" not in _CACHE:
        _CACHE["nc"] = _build()
    nc = _CACHE["nc"]

    BF = ml_dtypes.bfloat16
    # per-head query temperature 0.125*exp(-beta) folded into Wq/bq
    qscale = (0.125 * np.exp(-beta.astype(np.float64))).astype(np.float32)
    qs_hd = np.repeat(qscale, DH)                      # (D,) per out-dim
    Wq_s = Wq * qs_hd[:, None]
    bq_s = bq * qs_hd

    in_maps = []
    for core in range(8):
        b, hh = core // 4, core % 4
        hd = slice(hh * HD, hh * HD + HD)
        qct = np.zeros((64, HPC, C), BF)
        qc_r = q_c[:, hd].reshape(C, HPC, DH)          # (c, h, d)
        for h in range(HPC):
            qct[:, h, :] = qc_r[:, h, :].T.astype(BF)
        in_maps.append({
            "xT": np.ascontiguousarray(x[:, b, :].T.astype(BF)),
            "wqT": np.ascontiguousarray(Wq_s[hd, :].T.astype(BF)),
            "wkT": np.ascontiguousarray(Wk[hd, :].T.astype(BF)),
            "wvT": np.ascontiguousarray(Wv[hd, :].T.astype(BF)),
            "woT": np.ascontiguousarray(Wo[:, hd].T.astype(BF)),
            "qcT": qct,
            "bq": np.ascontiguousarray(bq_s[hd]),
            "bk": np.ascontiguousarray(bk[hd]),
            "bv": np.ascontiguousarray(bv[hd].astype(BF))[None, :],
            "bo4": np.ascontiguousarray((bo * 0.25).astype(BF))[None, :],
        })

    trace = os.environ.get("TRN_PROFILE", "0") == "1"
    res = run_bass_kernel_spmd(nc, in_maps, list(range(8)), trace=trace)
    LAST_EXEC_NS = res.exec_time_ns

    out = np.zeros((S, B, D), np.float32)
    for core in range(8):
        out[:, core // 4, :] += res.results[core]["out_p"]
    return out


# revision 17
# speedup vs baseline: 1.6684x; 1.2013x over previous
"""CompressionAttention Trainium2 kernel (8 NeuronCores, SPMD).

Sharding: core i handles batch b=i//4 and 4 heads hh=i%4 (model-dim slice
hh*256:(hh+1)*256). Heads never interact before out_proj, so each core
computes a partial out-projection for its batch; the host gather sums the
4 partials per batch and adds bo once.

Algorithm per core (chunked linear attention, chunk T=128):
  w[c,t] = exp(qc_c . k_t)            (max-subtraction dropped: att is
                                       invariant to per-c scaling of w)
  den[c,s]   = cumsum_t<=s w[c,t]
  att[c,s]   = (w_chunk^T (U o G) + SK^T qs) / den   per chunk + state
  p = softmax_c att ; o_s = sum_c p * (sum_t<=s w v)/den

Schedule: all cross-chunk recurrences (den carry, SK/SV prefixes) and all
G = k^T q blocks are precomputed in side phases, so the main chunk loop has
no chunk->chunk dependency and the Tensor engine never idles long enough to
HAM-throttle. Softmax elementwise ops run in a partition-packed (32h+c, t)
layout (engine time ~ free-size, so 4x cheaper than (c, 4h*t)); the packed
e/pdd are re-laid-out for their matmul consumers with tiny SBUF-to-SBUF
DMAs. 1/x runs as exp(-ln(x)) on Act (single act table, den carry folded
into the ln bias); 1/sums o-normalization is deferred out of the loop.
"""
import os
import numpy as np
import ml_dtypes

import concourse.bacc as bacc
import concourse.tile as tile
from concourse import mybir
from concourse.bass_utils import run_bass_kernel_spmd

F32 = mybir.dt.float32
BF16 = mybir.dt.bfloat16
EXP = mybir.ActivationFunctionType.Exp
LN = mybir.ActivationFunctionType.Ln
IDENT = mybir.ActivationFunctionType.Identity

S, B, D = 2048, 2, 1024
H, DH, C = 16, 64, 16
HPC = 4            # heads per core
HD = HPC * DH      # 256 model dims per core
T = 128            # chunk
NCH = S // T       # 16 chunks
KT = D // 128      # 8 k-tiles of the model dim

LAST_EXEC_NS = None
_CACHE = {}


def _patched_insert_act_table_loads(self):
    """Force every activation (Exp/Ln/Copy/Identity) onto the single
    act-func table that contains them all. The default greedy pass picks
    exp_and_others for Exp and natural_log_exp_and_others for Ln, inserting
    an ACT_TABLE_LOAD (1.28us!) on every switch."""
    import bass_rust as _bass_rust
    from concourse.hw_specs import get_activation_tables
    has_activation = any(
        isinstance(i, mybir.InstActivation)
        for b in self.main_func.blocks
        for i in b.instructions)
    if not has_activation:
        return
    keep = {mybir.ActivationFunctionType.Exp, mybir.ActivationFunctionType.Ln,
            mybir.ActivationFunctionType.Copy,
            mybir.ActivationFunctionType.Identity}
    tables = []
    for name, funcs in get_activation_tables(self.m.arch).items():
        if name != "natural_log_exp_and_others":
            funcs = funcs - keep
        tables.append((name, funcs))
    _bass_rust.insert_act_table_loads(self, tables)


def _build():
    nc = bacc.Bacc("TRN2", target_bir_lowering=False, debug=False, num_devices=8)
    import types as _types
    nc.insert_act_table_loads = _types.MethodType(
        _patched_insert_act_table_loads, nc)

    # ---- DRAM I/O (per-core, host-prepped layouts; bf16 where possible) ----
    xT_d = nc.dram_tensor("xT", [D, S], BF16, kind="ExternalInput")
    wqT_d = nc.dram_tensor("wqT", [D, HD], BF16, kind="ExternalInput")
    wkT_d = nc.dram_tensor("wkT", [D, HD], BF16, kind="ExternalInput")
    wvT_d = nc.dram_tensor("wvT", [D, HD], BF16, kind="ExternalInput")
    woT_d = nc.dram_tensor("woT", [HD, D], BF16, kind="ExternalInput")
    # qc transposed per head, zero-padded 16 -> 32 so the packed wT tile has
    # no uninitialized columns (pad logits are 0 -> w=1, never read).
    qcT_d = nc.dram_tensor("qcT", [64, HPC, 32], BF16, kind="ExternalInput")
    bq_d = nc.dram_tensor("bq", [HD], F32, kind="ExternalInput")
    bk_d = nc.dram_tensor("bk", [HD], F32, kind="ExternalInput")
    bv_d = nc.dram_tensor("bv", [1, HD], BF16, kind="ExternalInput")
    out_d = nc.dram_tensor("out_p", [S, D], F32, kind="ExternalOutput")

    # ---- consts baked into the NEFF ----
    u = np.triu(np.ones((T, T), np.float32))
    u4 = np.tile(u, (1, 4)).astype(ml_dtypes.bfloat16)
    u4_d = nc.inline_tensor(u4, "u4c")
    onesc64_d = nc.inline_tensor(np.ones((C, 64), ml_dtypes.bfloat16), "onesc64")
    onescol_d = nc.inline_tensor(np.ones((128, 1), ml_dtypes.bfloat16), "onescol")
    onesrow_d = nc.inline_tensor(np.ones((1, 128), ml_dtypes.bfloat16), "onesrow")
    kv_scratch = [nc.dram_tensor(f"kvs{j}", [128, 2048], BF16) for j in range(2)]

    with tile.TileContext(nc) as tc:
        _emit(nc, tc, locals())
    nc.compile()
    return nc


def _emit(nc, tc, d):
    from contextlib import ExitStack

    with ExitStack() as ctx:
        ep = ctx.enter_context

        # ---------- persistent pools ----------
        consts = ep(tc.tile_pool(name="consts", bufs=1))
        wpool = ep(tc.tile_pool(name="wpool", bufs=1))      # WoT
        qkv = ep(tc.tile_pool(name="qkv", bufs=1))          # qT/kT/vnat
        qkv2 = ep(tc.tile_pool(name="qkv2", bufs=1))        # knat, row-0 copies
        wstore = ep(tc.tile_pool(name="wstore", bufs=1))    # w16/wT/carry/sk/sv
        gstore = ep(tc.tile_pool(name="gstore", bufs=1))    # masked G, all chunks
        onum = ep(tc.tile_pool(name="onum", bufs=1))        # o_num, rb_all
        sb2 = ep(tc.tile_pool(name="sb2", bufs=2))          # per-chunk sbuf
        otp = ep(tc.tile_pool(name="otp", bufs=1))          # oT final
        outp = ep(tc.tile_pool(name="outp", bufs=3))        # out staging

        # ---------- consts ----------
        u4_t = consts.tile([128, 4 * T], BF16, tag="u4")
        nc.sync.dma_start(out=u4_t, in_=d["u4_d"].ap())
        u128 = u4_t[:, 0:T]
        onesc64_t = consts.tile([C, 64], BF16, tag="onesc64")
        nc.sync.dma_start(out=onesc64_t, in_=d["onesc64_d"].ap())
        onescol_t = consts.tile([128, 1], BF16, tag="onescol")
        nc.sync.dma_start(out=onescol_t, in_=d["onescol_d"].ap())
        onesrow_t = consts.tile([1, 128], BF16, tag="onesrow")
        nc.sync.dma_start(out=onesrow_t, in_=d["onesrow_d"].ap())
        qcT_t = consts.tile([64, HPC, 32], BF16, tag="qcT")
        nc.sync.dma_start(out=qcT_t, in_=d["qcT_d"].ap())
        bv_t = consts.tile([1, HD], BF16, tag="bv")
        nc.sync.dma_start(out=bv_t, in_=d["bv_d"].ap())

        # biases as (128, 2) per-partition columns (q/k only; v uses a row)
        def bias_tile(name):
            t = consts.tile([128, 2], F32, tag=name)
            nc.sync.dma_start(
                out=t, in_=d[name + "_d"].ap().rearrange("(m p) -> p m", p=128))
            return t
        bq_t, bk_t = bias_tile("bq"), bias_tile("bk")

        # WoT resident (bf16): (128, 2, D)
        woT_t = wpool.tile([128, 2, D], BF16, tag="woT")
        wor = d["woT_d"].ap().rearrange("(kt p) j -> kt p j", p=128)
        for kt in range(2):
            nc.sync.dma_start(out=woT_t[:, kt, :], in_=wor[kt])

        # ---------- projections (all bf16) ----------
        qT = [qkv.tile([128, S], BF16, tag=f"qT{m}", name=f"qT{m}") for m in range(2)]
        kT = [qkv.tile([128, S], BF16, tag=f"kT{m}", name=f"kT{m}") for m in range(2)]
        vnat = qkv.tile([128, NCH, HD], BF16, tag="vnat", name="vnat")

        with tc.tile_pool(name="xw", bufs=1) as xw, \
                tc.tile_pool(name="ppj", bufs=2, space="PSUM") as ppj:
            xT_t = xw.tile([128, KT, S], BF16, tag="xT")
            xr = d["xT_d"].ap().rearrange("(kt p) s -> kt p s", p=128)
            for kt in range(KT):
                nc.sync.dma_start(out=xT_t[:, kt, :], in_=xr[kt])
            wts = {}
            for nm in ("wq", "wk", "wv"):
                wt = xw.tile([128, KT, HD], BF16, tag=nm)
                wr = d[nm + "T_d"].ap().rearrange("(kt p) j -> kt p j", p=128)
                for kt in range(KT):
                    nc.sync.dma_start(out=wt[:, kt, :], in_=wr[kt])
                wts[nm] = wt

            epi = 0
            for nm, dst, bias in (("wq", qT, bq_t), ("wk", kT, bk_t)):
                for m in range(2):
                    for n in range(4):
                        ps = ppj.tile([128, 512], F32, tag="pj")
                        for kt in range(KT):
                            nc.tensor.matmul(
                                ps, wts[nm][:, kt, 128 * m:128 * m + 128],
                                xT_t[:, kt, 512 * n:512 * n + 512],
                                start=(kt == 0), stop=(kt == KT - 1))
                        if epi % 2 == 0:
                            nc.vector.tensor_scalar(
                                out=dst[m][:, 512 * n:512 * n + 512], in0=ps,
                                scalar1=bias[:, m:m + 1], scalar2=None,
                                op0=mybir.AluOpType.add)
                        else:
                            nc.scalar.activation(
                                out=dst[m][:, 512 * n:512 * n + 512], in_=ps,
                                func=IDENT, bias=bias[:, m:m + 1])
                        epi += 1

            # v directly in natural (s, hd) layout: out = xT.T @ wvT
            for sc in range(NCH):
                vp = ppj.tile([128, HD], F32, tag="vp")
                for kt in range(KT):
                    nc.tensor.matmul(
                        vp, xT_t[:, kt, T * sc:T * sc + T], wts["wv"][:, kt, :],
                        start=(kt == 0), stop=False)
                nc.tensor.matmul(vp, onesrow_t, bv_t, start=False, stop=True)
                if sc % 2 == 0:
                    nc.vector.tensor_copy(out=vnat[:, sc, :], in_=vp)
                else:
                    nc.scalar.copy(out=vnat[:, sc, :], in_=vp)

        # k natural layout via DMA xbar transpose, staged via DRAM
        # (SBUF-source xbar transpose is fatal on HW)
        knat = [qkv2.tile([128, NCH, 128], BF16, tag=f"knat{m}", name=f"knat{m}")
                for m in range(2)]
        kvs = d["kv_scratch"]
        for m in range(2):
            nc.sync.dma_start(out=kvs[m].ap(), in_=kT[m][:, :])
            nc.sync.dma_start_transpose(out=knat[m], in_=kvs[m].ap())

        # row-0 copies of odd heads' q/k (PE row-position changes between
        # back-to-back matmuls are fatal on HW; keep every operand at row 0)
        qTo = [qkv2.tile([64, S], BF16, tag=f"qTo{j}", name=f"qTo{j}") for j in range(2)]
        kTo = [qkv2.tile([64, S], BF16, tag=f"kTo{j}", name=f"kTo{j}") for j in range(2)]
        for j in range(2):
            nc.sync.dma_start(out=qTo[j], in_=qT[j][64:128, :])
            nc.sync.dma_start(out=kTo[j], in_=kT[j][64:128, :])

        def qTh(h):
            return qT[h // 2][0:64, :] if h % 2 == 0 else qTo[h // 2][:, :]

        def kTh(h):
            return kT[h // 2][0:64, :] if h % 2 == 0 else kTo[h // 2][:, :]

        # ---------- G = (k^T q) o U for all chunks (keeps PE dense) ----------
        gmt_all = gstore.tile([128, NCH, 4 * T], BF16, tag="gmt")
        with tc.tile_pool(name="pg", bufs=2, space="PSUM") as pg:
            for i in range(NCH):
                ch = slice(T * i, T * i + T)
                gt = pg.tile([128, 512], F32, tag="gt")
                for h in range(HPC):
                    nc.tensor.matmul(
                        gt[:, 128 * h:128 * h + 128],
                        kTh(h)[:, ch], qTh(h)[:, ch], start=True, stop=True)
                nc.vector.tensor_mul(gmt_all[:, i, :], gt, u4_t)

        # ---------- w16, packed wT, den carries, SK/SV prefixes ----------
        w16 = wstore.tile([C, HPC, S], BF16, tag="w16")
        # wT packed (t, 32h+c): cols 16-31 of each 32-group hold exp(0)=1
        # from the zero-padded qc -- written, never read back.
        wT_pad = wstore.tile([128, NCH, HPC, 32], BF16, tag="wTp")
        carry_cols = wstore.tile([128, NCH], F32, tag="carryc")
        sk_all = wstore.tile([64, NCH, HPC, C], BF16, tag="sk")
        sv_all = wstore.tile([C, NCH, HPC, DH], BF16, tag="sv")

        with tc.tile_pool(name="ph3a", bufs=2, space="PSUM") as ph3a:
            # w16 = exp(qc . k), free-packed (c, h, s)
            for n in range(4):
                ps = ph3a.tile([128, 512], F32, tag="w16ps")
                for h in range(HPC):
                    nc.tensor.matmul(
                        ps[32 * h:32 * h + C, :], qcT_t[:, h, 0:C],
                        kTh(h)[:, 512 * n:512 * n + 512],
                        start=True, stop=True, tile_position=(0, 32 * h))
                for h in range(HPC):
                    nc.scalar.activation(
                        out=w16[:, h, 512 * n:512 * n + 512],
                        in_=ps[32 * h:32 * h + C, :], func=EXP)

        with tc.tile_pool(name="ph3b", bufs=2, space="PSUM") as ph3b:
            # wT (t, 32h+c) = exp(k . qc_pad); cs -> den carry column chain
            cs_prev = None
            for i in range(NCH):
                ch = slice(T * i, T * i + T)
                tp = ph3b.tile([128, HPC, 32], F32, tag="tp")
                for h in range(HPC):
                    nc.tensor.matmul(
                        tp[:, h, :], kTh(h)[:, ch], qcT_t[:, h, :],
                        start=True, stop=True)
                nc.scalar.activation(out=wT_pad[:, i], in_=tp, func=EXP)
                if i < NCH - 1:
                    cs = ph3b.tile([128, 1], F32, tag="cs")
                    nc.tensor.matmul(cs, wT_pad[:, i], onescol_t,
                                     start=True, stop=True)
                    if cs_prev is None:
                        nc.scalar.copy(out=carry_cols[:, 1:2], in_=cs)
                    else:
                        nc.vector.tensor_add(carry_cols[:, i + 1:i + 2], cs,
                                             carry_cols[:, i:i + 1])
                    cs_prev = cs

        with tc.tile_pool(name="ph3c", bufs=2, space="PSUM") as ph3c:
            # SK/SV chunk deltas + exclusive prefixes (slot i = state
            # before chunk i; slot 0 unused)
            skd_prev = None
            for i in range(NCH - 1):
                skd = ph3c.tile([64, HPC, C], F32, tag="skd")
                svd = ph3c.tile([C, HPC, DH], F32, tag="svd")
                for h in range(HPC):
                    nc.tensor.matmul(
                        skd[:, h, :],
                        knat[h // 2][:, i, 64 * (h % 2):64 * (h % 2) + 64],
                        wT_pad[:, i, h, 0:C], start=True, stop=True)
                    nc.tensor.matmul(
                        svd[:, h, :], wT_pad[:, i, h, 0:C],
                        vnat[:, i, 64 * h:64 * h + 64], start=True, stop=True)
                if skd_prev is None:
                    nc.scalar.copy(out=sk_all[:, 1], in_=skd)
                    nc.scalar.copy(out=sv_all[:, 1], in_=svd)
                else:
                    nc.vector.tensor_add(sk_all[:, i + 1], skd, sk_all[:, i])
                    nc.vector.tensor_add(sv_all[:, i + 1], svd, sv_all[:, i])
                skd_prev = skd

        # ---------- main chunk loop (no cross-chunk dependencies) ----------
        o_nm = onum.tile([128, 2, S], BF16, tag="o_nm")
        rb_all = onum.tile([128, 2, S], BF16, tag="rb_all")

        pden = ep(tc.tile_pool(name="pden", bufs=2, space="PSUM"))
        pan = ep(tc.tile_pool(name="pan", bufs=2, space="PSUM"))
        ppw = ep(tc.tile_pool(name="ppw", bufs=2, space="PSUM"))
        pmr = ep(tc.tile_pool(name="pmr", bufs=2, space="PSUM"))

        for i in range(NCH):
            ch = slice(T * i, T * i + T)

            # den (packed 32h+c) in ONE matmul; att numerator per head into
            # the same packed layout via tile_position column offsets.
            den_b = pden.tile([128, T], F32, tag="den")
            nc.tensor.matmul(den_b, wT_pad[:, i], u128, start=True, stop=True)
            an_b = pan.tile([128, T], F32, tag="an")
            for h in range(HPC):
                nc.tensor.matmul(
                    an_b[32 * h:32 * h + 32, :], wT_pad[:, i, h, :],
                    gmt_all[:, i, 128 * h:128 * h + 128],
                    start=True, stop=(i == 0), tile_position=(0, 32 * h))
                if i > 0:
                    nc.tensor.matmul(
                        an_b[32 * h:32 * h + C, :], sk_all[:, i, h, :],
                        qTh(h)[:, ch], start=False, stop=True,
                        tile_position=(0, 32 * h))

            # softmax pieces, packed layout: 1/x as exp(-ln x), den carry
            # folded into the ln bias (per-partition column).
            lden = sb2.tile([128, T], F32, tag="lden")
            if i == 0:
                nc.scalar.activation(out=lden, in_=den_b, func=LN)
            else:
                nc.scalar.activation(out=lden, in_=den_b, func=LN,
                                     bias=carry_cols[:, i:i + 1])
            rden = sb2.tile([128, T], F32, tag="rden")
            nc.scalar.activation(out=rden, in_=lden, func=EXP, scale=-1.0)
            att = sb2.tile([128, T], F32, tag="att")
            nc.vector.tensor_mul(att, an_b, rden)
            e_b = sb2.tile([128, T], BF16, tag="e")
            nc.scalar.activation(out=e_b, in_=att, func=EXP)
            pdd_b = sb2.tile([128, T], BF16, tag="pdd")
            nc.gpsimd.tensor_mul(pdd_b, e_b, rden)

            # re-layout packed (32h+c, t) -> (c, h, t) for matmul consumers
            e_u = sb2.tile([C, HPC, T], BF16, tag="eu")
            pdd_u = sb2.tile([C, HPC, T], BF16, tag="pddu")
            for h in range(HPC):
                nc.sync.dma_start(out=e_u[:, h, :],
                                  in_=e_b[32 * h:32 * h + C, :])
                nc.sync.dma_start(out=pdd_u[:, h, :],
                                  in_=pdd_b[32 * h:32 * h + C, :])

            # PW (t, s) + mask
            pw = ppw.tile([128, 512], F32, tag="pw")
            for h in range(HPC):
                nc.tensor.matmul(
                    pw[:, 128 * h:128 * h + 128], w16[:, h, ch],
                    pdd_u[:, h, :], start=True, stop=True)
            pwm = sb2.tile([128, 512], BF16, tag="pwm")
            nc.vector.tensor_mul(pwm, pw, u4_t)

            # o numerator (cols 0-255) + sums broadcast (cols 256-511)
            mr = pmr.tile([128, 512], F32, tag="mr")
            for h in range(HPC):
                bp = 64 * (h % 2)
                cb = 128 * (h // 2)
                nc.tensor.matmul(
                    mr[bp:bp + 64, cb:cb + 128],
                    vnat[:, i, 64 * h:64 * h + 64],
                    pwm[:, 128 * h:128 * h + 128],
                    start=True, stop=(i == 0), tile_position=(0, bp))
                if i > 0:
                    nc.tensor.matmul(
                        mr[bp:bp + 64, cb:cb + 128],
                        sv_all[:, i, h, :], pdd_u[:, h, :],
                        start=False, stop=True, tile_position=(0, bp))
                nc.tensor.matmul(
                    mr[bp:bp + 64, 256 + cb:256 + cb + 128],
                    onesc64_t, e_u[:, h, :],
                    start=True, stop=True, tile_position=(0, bp))
            nc.vector.tensor_copy(out=o_nm[:, :, ch], in_=mr[:, 0:256])
            lsb = sb2.tile([128, 256], F32, tag="lsb")
            nc.scalar.activation(out=lsb, in_=mr[:, 256:512], func=LN)
            nc.scalar.activation(out=rb_all[:, :, ch], in_=lsb,
                                 func=EXP, scale=-1.0)

        # ---------- tail: deferred softmax normalization + out-proj ----------
        oT = [otp.tile([128, S], BF16, tag=f"oT{m}", name=f"oT{m}")
              for m in range(2)]
        for m in range(2):
            nc.vector.tensor_mul(oT[m], o_nm[:, m, :], rb_all[:, m, :])

        for sc in range(NCH):
            ch = slice(T * sc, T * sc + T)
            ob = outp.tile([128, D], F32, tag="ob")
            for n2 in range(2):
                ps = pmr.tile([128, 512], F32, tag="mr")
                for kt in range(2):
                    nc.tensor.matmul(
                        ps, oT[kt][:, ch],
                        woT_t[:, kt, 512 * n2:512 * n2 + 512],
                        start=(kt == 0), stop=(kt == 1))
                if n2 == 0:
                    nc.vector.tensor_copy(
                        out=ob[:, 512 * n2:512 * n2 + 512], in_=ps)
                else:
                    nc.scalar.copy(
                        out=ob[:, 512 * n2:512 * n2 + 512], in_=ps)
            nc.sync.dma_start(out=d["out_d"].ap()[ch, :], in_=ob)


def kernel(**inputs):
    global LAST_EXEC_NS
    x = np.ascontiguousarray(inputs["x"], np.float32)
    q_c, beta = np.asarray(inputs["q_c"]), np.asarray(inputs["beta"])
    Wq, bq = np.asarray(inputs["Wq"]), np.asarray(inputs["bq"])
    Wk, bk = np.asarray(inputs["Wk"]), np.asarray(inputs["bk"])
    Wv, bv = np.asarray(inputs["Wv"]), np.asarray(inputs["bv"])
    Wo, bo = np.asarray(inputs["Wo"]), np.asarray(inputs["bo"])

    if "nc" not in _CACHE:
        _CACHE["nc"] = _build()
    nc = _CACHE["nc"]

    BF = ml_dtypes.bfloat16
    # per-head query temperature 0.125*exp(-beta) folded into Wq/bq
    qscale = (0.125 * np.exp(-beta.astype(np.float64))).astype(np.float32)
    qs_hd = np.repeat(qscale, DH)                      # (D,) per out-dim
    Wq_s = Wq * qs_hd[:, None]
    bq_s = bq * qs_hd

    in_maps = []
    for core in range(8):
        b, hh = core // 4, core % 4
        hd = slice(hh * HD, hh * HD + HD)
        qct = np.zeros((64, HPC, 32), BF)
        qc_r = q_c[:, hd].reshape(C, HPC, DH)          # (c, h, d)
        for h in range(HPC):
            qct[:, h, 0:C] = qc_r[:, h, :].T.astype(BF)
        in_maps.append({
            "xT": np.ascontiguousarray(x[:, b, :].T.astype(BF)),
            "wqT": np.ascontiguousarray(Wq_s[hd, :].T.astype(BF)),
            "wkT": np.ascontiguousarray(Wk[hd, :].T.astype(BF)),
            "wvT": np.ascontiguousarray(Wv[hd, :].T.astype(BF)),
            "woT": np.ascontiguousarray(Wo[:, hd].T.astype(BF)),
            "qcT": qct,
            "bq": np.ascontiguousarray(bq_s[hd]),
            "bk": np.ascontiguousarray(bk[hd]),
            "bv": np.ascontiguousarray(bv[hd].astype(BF))[None, :],
        })

    trace = os.environ.get("TRN_PROFILE", "0") == "1"
    res = run_bass_kernel_spmd(nc, in_maps, list(range(8)), trace=trace)
    LAST_EXEC_NS = res.exec_time_ns

    out = np.zeros((S, B, D), np.float32)
    for core in range(8):
        out[:, core // 4, :] += res.results[core]["out_p"]
    out += bo[None, None, :].astype(np.float32)
    return out
